# revision 4
# baseline (speedup 1.0000x reference)
"""Channel-attention kernel for Trainium2 (8 NeuronCores, SPMD data-parallel).

out[b] = beta * softmax(rowmax(S) - S, axis=-1) @ x[b] + x[b],  S = x[b] @ x[b].T

Sharding: batch dim B=16 split as 2 batches per core across 8 cores.
Each core gets x (natural fp32, for the A@x matmul + exact epilogue add)
and a host-pretransposed bf16 copy xT (the S matmul contracts over the
feature dim n, which must live on SBUF partitions for the PE).

Math note: softmax(max_row - S) row-wise equals exp(minrow - S) / Z with
Z = sum_d exp(minrow - S).  We fold beta/Z into A before the second
matmul, so when beta == 0 the kernel returns x bit-exactly.
"""

from contextlib import ExitStack

import numpy as np
import ml_dtypes

N_CORES = 8
B, C, N = 16, 512, 4096
BPC = B // N_CORES  # batches per core
P = 128
MT = C // P  # 4 row-blocks of channels
KT = N // P  # 32 partition-tiles of xT
NQ = N // 512  # 8 n-chunks for the second matmul
KD = C // P  # 4 d-chunks for the second matmul

_CACHE = {}


def _build_bass(reps=1):
    import concourse.bass as bass
    import concourse.bacc as bacc
    import concourse.mybir as mybir
    from concourse import tile, masks

    dt = mybir.dt
    AF = mybir.ActivationFunctionType
    ALU = mybir.AluOpType
    AX = mybir.AxisListType

    nc = bacc.Bacc(
        "TRN2", target_bir_lowering=False, debug=False, num_devices=N_CORES
    )

    x_dram = nc.dram_tensor("x", [BPC, C, N], dt.float32, kind="ExternalInput")
    xt_dram = nc.dram_tensor("xt", [BPC, N, C], dt.bfloat16, kind="ExternalInput")
    beta_dram = nc.dram_tensor("beta", [1, 1], dt.float32, kind="ExternalInput")
    out_dram = nc.dram_tensor("out", [BPC, C, N], dt.float32, kind="ExternalOutput")

    with tile.TileContext(nc) as tc, ExitStack() as ctx:
        const_pool = ctx.enter_context(tc.tile_pool(name="const", bufs=1))
        x_pool = ctx.enter_context(tc.tile_pool(name="x", bufs=4))
        xt_pool = ctx.enter_context(tc.tile_pool(name="xt", bufs=2))
        xb_pool = ctx.enter_context(tc.tile_pool(name="xb", bufs=1))
        a_pool = ctx.enter_context(tc.tile_pool(name="a", bufs=2))
        at_pool = ctx.enter_context(tc.tile_pool(name="at", bufs=2))
        st_pool = ctx.enter_context(tc.tile_pool(name="st", bufs=2))
        spsum = ctx.enter_context(
            tc.tile_pool(name="spsum", bufs=4, space=bass.MemorySpace.PSUM)
        )
        tpsum = ctx.enter_context(
            tc.tile_pool(name="tpsum", bufs=2, space=bass.MemorySpace.PSUM)
        )
        fpsum = ctx.enter_context(
            tc.tile_pool(name="fpsum", bufs=2, space=bass.MemorySpace.PSUM)
        )

        ident = const_pool.tile([P, P], dt.bfloat16)
        masks.make_identity(nc, ident[:])

        # Broadcast beta scalar to all 128 partitions via ones.T @ beta.
        ones = const_pool.tile([1, P], dt.float32)
        nc.gpsimd.memset(ones[:], 1.0)
        beta_sb = const_pool.tile([1, 1], dt.float32)
        nc.sync.dma_start(beta_sb[:], beta_dram[:])
        beta_ps = spsum.tile([P, 1], dt.float32, tag="s_ps")
        nc.tensor.matmul(beta_ps[:], ones[:], beta_sb[:], start=True, stop=True)
        beta128 = const_pool.tile([P, 1], dt.float32)
        nc.scalar.copy(beta128[:], beta_ps[:])

        for b in [b for _ in range(reps) for b in range(BPC)]:
            # ---- loads ----
            xt_sb = xt_pool.tile([P, KT, 512], dt.bfloat16)
            xt_src = xt_dram[b].rearrange("(k p) c -> p k c", p=P)
            for g in range(8):
                nc.sync.dma_start(
                    xt_sb[:, 4 * g : 4 * g + 4, :], xt_src[:, 4 * g : 4 * g + 4, :]
                )
            x_src = x_dram[b].rearrange("(m p) n -> p m n", p=P)
            x_tiles = []
            for m in range(MT):
                xt_t = x_pool.tile([P, N], dt.float32, tag="x")
                for g in range(2):
                    nc.sync.dma_start(
                        xt_t[:, 2048 * g : 2048 * (g + 1)],
                        x_src[:, m, 2048 * g : 2048 * (g + 1)],
                    )
                x_tiles.append(xt_t)

            # ---- cast x -> bf16 for the A @ x matmul's moving operand ----
            xb = xb_pool.tile([P, MT, N], dt.bfloat16)
            for m in range(MT):
                nc.scalar.copy(xb[:, m, :], x_tiles[m][:])

            # ---- S = x @ x.T  (contraction over n on partitions) ----
            s_tiles = []
            for m in range(MT):
                s_ps = spsum.tile([P, 512], dt.float32, tag="s_ps")
                for k in range(KT):
                    nc.tensor.matmul(
                        s_ps[:],
                        xt_sb[:, k, P * m : P * (m + 1)],
                        xt_sb[:, k, :],
                        start=(k == 0),
                        stop=(k == KT - 1),
                    )
                s_tiles.append(s_ps)

            # ---- softmax: A = exp(minrow - S) * (beta / Z) ----
            a_sb = a_pool.tile([P, MT, 512], dt.bfloat16)
            minr = st_pool.tile([P, MT], dt.float32, tag="minr")
            zsum = st_pool.tile([P, MT], dt.float32, tag="z")
            rzb = st_pool.tile([P, MT], dt.float32, tag="rzb")
            for m in range(MT):
                nc.vector.tensor_reduce(
                    minr[:, m : m + 1], s_tiles[m][:], axis=AX.X, op=ALU.min
                )
                nc.scalar.activation(
                    a_sb[:, m, :],
                    s_tiles[m][:],
                    AF.Exp,
                    bias=minr[:, m : m + 1],
                    scale=-1.0,
                    accum_out=zsum[:, m : m + 1],
                )
                nc.vector.reciprocal(rzb[:, m : m + 1], zsum[:, m : m + 1])
                nc.vector.tensor_mul(
                    rzb[:, m : m + 1], rzb[:, m : m + 1], beta128[:]
                )
                nc.vector.tensor_scalar_mul(
                    a_sb[:, m, :], a_sb[:, m, :], rzb[:, m : m + 1]
                )

            # ---- transpose A (16 PE-mode 128x128 transposes) ----
            at_sb = at_pool.tile([P, KD, 512], dt.bfloat16)
            for i in range(MT):
                for j in range(KD):
                    t_ps = tpsum.tile([P, P], dt.bfloat16, tag="t_ps")
                    nc.tensor.transpose(
                        t_ps[:], a_sb[:, i, P * j : P * (j + 1)], ident[:]
                    )
                    nc.scalar.copy(at_sb[:, j, P * i : P * (i + 1)], t_ps[:])

            # ---- F = A @ x, epilogue out = F + x (in place), store ----
            out_dst = out_dram[b].rearrange("(m p) n -> p m n", p=P)
            for m in range(MT):
                for q in range(NQ):
                    f_ps = fpsum.tile([P, 512], dt.float32, tag="f_ps")
                    for kd in range(KD):
                        nc.tensor.matmul(
                            f_ps[:],
                            at_sb[:, kd, P * m : P * (m + 1)],
                            xb[:, kd, 512 * q : 512 * (q + 1)],
                            start=(kd == 0),
                            stop=(kd == KD - 1),
                        )
                    nc.vector.tensor_add(
                        x_tiles[m][:, 512 * q : 512 * (q + 1)],
                        f_ps[:],
                        x_tiles[m][:, 512 * q : 512 * (q + 1)],
                    )
                for g in range(2):
                    nc.sync.dma_start(
                        out_dst[:, m, 2048 * g : 2048 * (g + 1)],
                        x_tiles[m][:, 2048 * g : 2048 * (g + 1)],
                    )

    nc.compile()
    return nc


def _get_nc(reps=1):
    key = ("nc", reps)
    if key not in _CACHE:
        _CACHE[key] = _build_bass(reps)
    return _CACHE[key]


def _make_in_maps(x, beta):
    x = np.ascontiguousarray(x, dtype=np.float32)
    xt16 = np.ascontiguousarray(
        x.transpose(0, 2, 1), dtype=np.float32
    ).astype(ml_dtypes.bfloat16)
    beta_arr = np.asarray(beta, dtype=np.float32).reshape(1, 1)
    in_maps = []
    for i in range(N_CORES):
        sl = slice(BPC * i, BPC * (i + 1))
        in_maps.append(
            {
                "x": np.ascontiguousarray(x[sl]),
                "xt": np.ascontiguousarray(xt16[sl]),
                "beta": beta_arr,
            }
        )
    return in_maps


def _run(x, beta, trace=False, **kwargs):
    from concourse.bass_utils import run_bass_kernel_spmd

    nc = _get_nc()
    in_maps = _make_in_maps(x, beta)
    res = run_bass_kernel_spmd(
        nc, in_maps, core_ids=list(range(N_CORES)), trace=trace, **kwargs
    )
    out = np.concatenate([np.asarray(r["out"]) for r in res.results], axis=0)
    return out.astype(np.float32, copy=False), res


def kernel(x, beta):
    out, _ = _run(np.asarray(x), np.asarray(beta))
    return out
